# revision 23
# baseline (speedup 1.0000x reference)
"""Causal self-attention (B=1, T=4096, C=768, H=12, hd=64) on 8 trn2 NeuronCores.

Strategy (all FLOPs on device, host only reshapes/slices):
  Launch 1 (sequence-parallel): core c computes qkv for rows [512c, 512c+512):
    q^T, k^T produced directly in [channel, pos] layout via W^T @ x^T (fp32),
    with RoPE applied as  rope(a)^T = a*cosT + P(a*sinT)  where a = W^T x^T + b
    and P is the signed 64-block rotate-half permutation applied as a single
    128x128 matmul (sin is 32-periodic along the channel axis, so the same sin
    table works before the permutation).  v produced in natural [pos, channel]
    layout.  Outputs in bf16.
  Launch 2 (query-block-parallel): core c owns 4 query blocks of 128 rows
    [31-c, 16+c, 15-c, c] (padded causal kv-tile counts 32/24/16/8 -- identical
    SPMD program on every core).  All streams bf16.  Scores are computed
    transposed S^T[kv, q] so no transposes are needed anywhere; causal/padding
    masks are rank-4 augmentations of the contraction (4 extra "mask channels"
    in q^T/k^T); the per-block diagonal kv tile is processed separately with a
    constant triangular additive mask.  exp via ScalarE (no row-max needed:
    scores are N(0,1)-scale) emitting bf16 probs; denominator via an appended
    ones-column on V, per-head normalization via a PE-broadcast reciprocal,
    then the output projection contracts y^T directly (no transpose), bias
    b_proj' = b_proj + bv @ w_proj folded on host.  The kv schedule tapers:
    tiles 0-7 run 512 wide (4 query slots), 8-15 384 wide, 16-23 256 wide,
    24-30 128 wide.
"""

import numpy as np

import concourse.bass as bass
import concourse.bacc as bacc
import concourse.tile as tile
from concourse import mybir
from concourse.bass_utils import run_bass_kernel_spmd

F32 = mybir.dt.float32
F32R = mybir.dt.float32r
BF16 = mybir.dt.bfloat16
NPBF16 = mybir.dt.np(mybir.dt.bfloat16)

T, C, H, HD = 4096, 768, 12, 64
NCORES = 8
RPC = T // NCORES          # rows per core in launch 1 (512)
NT = T // 128              # kv tiles (32)
MASK = -2000.0             # additive mask; *0.125 -> exp underflows to 0
ROPE_BASE = 10000.0

# bf16-Schraudolph exp on the DVE: bf16_bits(exp(0.125*x)) ~ int16(x*S + B).
# B calibrated numerically for max relative error ~3.3% over raw-score range
# [-130, 48]; masked scores (-2000) land at tiny negative bf16 values (~0).
SCHRA_S = float(0.125 * np.log2(np.e) * 128)
SCHRA_B = 16250.5
# score groups whose exp runs on the DVE instead of ScalarE (plus the diag)
OFF_DVE = (1, 3, 5)

# launch-2 slot structure: slot s of core c handles the 64-row query block
# BLOCKS[c][s] (block b covers q rows [64b, 64b+64)).  Full (strictly
# below-diagonal) kv-tile count of block b is b//2; with b = 8*(7-s)+c the
# padded per-slot counts PADF = [31-4s] are uniform across cores.
BLOCKS = [[8 * (7 - s) + c for s in range(8)] for c in range(NCORES)]
PADF = [31 - 4 * s for s in range(8)]
# main-loop score groups: lists of (kv_tile, psum_col_offset, q_width);
# every tile's q columns are the prefix [0, width).  Offsets are placed so
# no matmul output crosses a 512-col PSUM bank boundary.
G64 = [
    [(0, 0, 512), (1, 512, 512)],
    [(2, 0, 512), (3, 512, 448)],
    [(4, 0, 448), (5, 512, 448)],
    [(6, 0, 448), (7, 512, 384)],
    [(8, 0, 384), (9, 512, 384)],
    [(10, 0, 384), (11, 512, 320)],
    [(12, 0, 320), (13, 512, 320)],
    [(14, 0, 320), (15, 512, 256), (16, 768, 256)],
    [(17, 0, 256), (18, 256, 256), (19, 512, 192), (20, 704, 192)],
    [(21, 0, 192), (22, 192, 192), (23, 384, 128), (24, 512, 128),
     (25, 640, 128), (26, 768, 128), (27, 896, 64), (28, 960, 64)],
    [(29, 0, 64), (30, 64, 64)],
]

HINTS = (mybir.EngineType.PE, mybir.EngineType.Activation, mybir.EngineType.DVE,
         mybir.EngineType.SP)


def _build_l1(reps=1):
    nc = bacc.Bacc("TRN2", target_bir_lowering=False, debug=False,
                   num_devices=NCORES)
    XT = nc.dram_tensor("xt", [C, RPC], F32R, kind="ExternalInput")
    WA = nc.dram_tensor("wa", [C, 3 * C], F32R, kind="ExternalInput")
    BQK = nc.dram_tensor("bqk", [128, 12], F32, kind="ExternalInput")
    PM = nc.dram_tensor("pm", [128, 128], BF16, kind="ExternalInput")
    COS = nc.dram_tensor("cos", [128, RPC], BF16, kind="ExternalInput")
    SIN = nc.dram_tensor("sin", [128, RPC], BF16, kind="ExternalInput")
    QKT = nc.dram_tensor("qkt", [2 * C, RPC], BF16, kind="ExternalOutput")
    VO = nc.dram_tensor("vo", [RPC, C], BF16, kind="ExternalOutput")

    with tile.TileContext(nc) as tc:
        with (
            tc.tile_pool(name="singles", bufs=1) as singles,
            tc.tile_pool(name="tmp", bufs=3) as tmp,
            tc.tile_pool(name="ps", bufs=2, space="PSUM") as ps,
            tc.tile_pool(name="psb", bufs=2, space="PSUM") as psb,
        ):
            wa_r = WA.rearrange("(k p) n -> p k n", p=128)
            xt_r = XT.rearrange("(k p) n -> p k n", p=128)
            xt_t = []
            for k in range(6):
                xk = singles.tile([128, RPC], F32R, tag=f"xt{k}")
                nc.sync.dma_start(out=xk, in_=xt_r[:, k, :])
                xt_t.append(xk)
            wa_t = []
            for m in range(12):
                wam = singles.tile([128, 6, 128], F32R, tag=f"wa{m}")
                nc.sync.dma_start(out=wam,
                                  in_=wa_r[:, :, 128 * m:128 * (m + 1)])
                wa_t.append(wam)
            wv_sb = singles.tile([128, 6, C], F32R)
            for k in range(6):
                nc.sync.dma_start(out=wv_sb[:, k, :],
                                  in_=wa_r[:, k, 2 * C:3 * C])
            bqk_sb = singles.tile([128, 12], F32)
            nc.sync.dma_start(out=bqk_sb, in_=BQK[:])
            pm_sb = singles.tile([128, 128], BF16)
            nc.sync.dma_start(out=pm_sb, in_=PM[:])
            cos_sb = singles.tile([128, RPC], BF16)
            nc.sync.dma_start(out=cos_sb, in_=COS[:])
            sin_sb = singles.tile([128, RPC], BF16)
            nc.sync.dma_start(out=sin_sb, in_=SIN[:])

            def body(_=None):
                # q^T, k^T with RoPE: 12 channel tiles of 128, stores paired
                o_sb = None
                for m in range(12):
                    ps_a = ps.tile([128, RPC], F32, tag="psa")
                    for k in range(6):
                        nc.tensor.matmul(
                            ps_a, wa_t[m][:, k, :],
                            xt_t[k], start=(k == 0), stop=(k == 5))
                    a_sb = tmp.tile([128, RPC], BF16, tag="a")
                    nc.scalar.activation(a_sb, ps_a,
                                         mybir.ActivationFunctionType.Identity,
                                         bias=bqk_sb[:, m:m + 1])
                    u_sb = tmp.tile([128, RPC], BF16, tag="u")
                    nc.vector.tensor_mul(u_sb, a_sb, sin_sb)
                    ps_b = psb.tile([128, RPC], F32, tag="psb")
                    nc.tensor.matmul(ps_b, pm_sb, u_sb, start=True, stop=True)
                    t1 = tmp.tile([128, RPC], BF16, tag="t1")
                    nc.vector.tensor_mul(t1, a_sb, cos_sb)
                    b_sb = tmp.tile([128, RPC], BF16, tag="b")
                    nc.scalar.copy(b_sb, ps_b)
                    if m % 2 == 0:
                        o_sb = tmp.tile([128, 2, RPC], BF16, tag="o")
                    nc.vector.tensor_add(o_sb[:, m % 2, :], t1, b_sb)
                    if m % 2 == 1:
                        nc.sync.dma_start(
                            out=QKT.rearrange("(a p) n -> p a n", p=128)
                            [:, m - 1:m + 1, :], in_=o_sb)

                # v in natural layout: 4 row tiles x (512 + 256) cols
                for qt in range(4):
                    vo_sb = tmp.tile([128, C], BF16, tag="vo")
                    for n0, nw in ((0, 512), (512, 256)):
                        ps_v = ps.tile([128, 512], F32, tag="psv")
                        for k in range(6):
                            nc.tensor.matmul(
                                ps_v[:, :nw],
                                xt_t[k][:, 128 * qt:128 * (qt + 1)],
                                wv_sb[:, k, n0:n0 + nw],
                                start=(k == 0), stop=(k == 5))
                        nc.vector.tensor_copy(vo_sb[:, n0:n0 + nw],
                                              ps_v[:, :nw])
                    nc.sync.dma_start(
                        out=VO[128 * qt:128 * (qt + 1), :], in_=vo_sb)

            if reps == 1:
                body()
            else:
                with tc.For_i(0, reps, 1, hint_engines=HINTS):
                    body()
    nc.finalize()
    return nc


def _build_l2(reps=1):
    nc = bacc.Bacc("TRN2", target_bir_lowering=False, debug=False,
                   num_devices=NCORES)
    KTM = nc.dram_tensor("ktm", [H, 72, T], BF16, kind="ExternalInput")
    QTM = nc.dram_tensor("qtm", [H, 72, 512], BF16, kind="ExternalInput")
    VP = nc.dram_tensor("vp", [H, 128, NT * (HD + 1)], BF16, kind="ExternalInput")
    KTD = nc.dram_tensor("ktd", [C, 1024], BF16, kind="ExternalInput")
    VD = nc.dram_tensor("vd", [H, 128, 8 * (HD + 1)], BF16, kind="ExternalInput")
    TRI = nc.dram_tensor("tri", [128, 512], F32, kind="ExternalInput")
    WP = nc.dram_tensor("wp", [C, C], BF16, kind="ExternalInput")
    ONESR = nc.dram_tensor("onesr", [1, 64], F32R, kind="ExternalInput")
    BP = nc.dram_tensor("bp", [1, C], F32, kind="ExternalInput")
    OUT = nc.dram_tensor("out", [512, C], F32, kind="ExternalOutput")

    with tile.TileContext(nc) as tc:
        with (
            tc.tile_pool(name="singles", bufs=1) as singles,
            tc.tile_pool(name="big", bufs=3) as big,
            tc.tile_pool(name="pt", bufs=7) as ptp,
            tc.tile_pool(name="small", bufs=3) as small,
            tc.tile_pool(name="sp", bufs=2, space="PSUM") as sp,
            tc.tile_pool(name="yp", bufs=3, space="PSUM") as yp,
            tc.tile_pool(name="rp", bufs=1, space="PSUM") as rp,
        ):
            # small one-time loads on the SWDGE queue so they don't
            # head-block the per-head HWDGE loads; the big wp load is deferred
            # into body() after head 0's loads (only needed at the proj tail)
            wp_sb = singles.tile([128, 6, C], BF16)
            tri_sb = singles.tile([128, 512], F32)
            nc.gpsimd.dma_start(out=tri_sb, in_=TRI[:])
            bp_sb = singles.tile([128, C], F32)
            ones65 = singles.tile([65, 64], F32R)
            nc.gpsimd.dma_start(out=ones65[64:65, :], in_=ONESR[:])
            yt_sb = singles.tile([128, 6, 512], BF16)

            def load_head(h):
                kth = big.tile([72, T], BF16, tag="kth")
                nc.sync.dma_start(out=kth, in_=KTM[h])
                qth = small.tile([72, 512], BF16, tag="qth")
                nc.gpsimd.dma_start(out=qth, in_=QTM[h])
                vh = big.tile([128, NT, HD + 1], BF16, tag="vh")
                nc.gpsimd.dma_start(out=vh, in_=VP[h])
                ktd = small.tile([64, 1024], BF16, tag="ktd")
                nc.gpsimd.dma_start(out=ktd, in_=KTD[64 * h:64 * (h + 1), :])
                vd_sb = small.tile([128, 8, HD + 1], BF16, tag="vd")
                nc.gpsimd.dma_start(out=vd_sb, in_=VD[h])
                return kth, qth, vh, ktd, vd_sb

            DEPTH = 3

            def compute_head_gen(h, tiles, prev_norm):
                """Generator: one yield per score group (12 total: 11 main +
                diag), so two heads' streams can be interleaved in lockstep.
                Returns the deferred-normalization closure."""
                kth, qth, vh, ktd, vd_sb = tiles
                state = {"y": None}
                pending = []

                def do_av(pgrp, ppt):
                    if state["y"] is None:
                        state["y"] = yp.tile([65, 512], F32, tag="y")
                    y_ps = state["y"]
                    for t, off, w, ds in pgrp:
                        if ds is None:
                            nc.tensor.matmul(
                                y_ps[:, 0:w], vh[:, t, :], ppt[:, off:off + w],
                                start=(t == 0), stop=False,
                                skip_group_check=True)
                        else:
                            nc.tensor.matmul(
                                y_ps[:, off:off + 64], vd_sb[:, ds, :],
                                ppt[:, off:off + 64],
                                start=False, stop=(ds == 7),
                                skip_group_check=True)

                for gi in range(12):
                    s2 = sp.tile([128, 1024], F32, tag="s2")
                    pt2 = ptp.tile([128, 1024], BF16, tag="pt2")
                    if gi < 11:
                        grp = [(t, off, w, None) for t, off, w in G64[gi]]
                        span = max(off + w for _, off, w in G64[gi])
                        for t, off, w in G64[gi]:
                            nc.tensor.matmul(
                                s2[:, off:off + w],
                                kth[:, 128 * t:128 * (t + 1)],
                                qth[:, 0:w],
                                start=True, stop=True)
                    else:
                        # diagonal kv tiles: one [128kv x 64q] block per slot,
                        # triangular causal mask added in place
                        grp = [(None, 64 * s, 64, s) for s in range(8)]
                        span = 512
                        for s in range(8):
                            nc.tensor.matmul(
                                s2[:, 64 * s:64 * (s + 1)],
                                ktd[:, 128 * s:128 * (s + 1)],
                                qth[0:64, 64 * s:64 * (s + 1)],
                                start=True, stop=True)
                        nc.vector.tensor_add(s2[:, 0:512], s2[:, 0:512],
                                             tri_sb)
                    if len(pending) >= DEPTH:
                        do_av(*pending.pop(0))
                    if gi in OFF_DVE or gi == 11:
                        nc.vector.tensor_scalar(
                            pt2[:, 0:span].bitcast(mybir.dt.int16),
                            s2[:, 0:span], SCHRA_S, SCHRA_B,
                            mybir.AluOpType.mult, mybir.AluOpType.add)
                    else:
                        nc.scalar.activation(pt2[:, 0:span], s2[:, 0:span],
                                             mybir.ActivationFunctionType.Exp,
                                             scale=0.125)
                    if gi == 2 and prev_norm is not None:
                        prev_norm()
                    pending.append((grp, pt2))
                    yield
                for p in pending:
                    do_av(*p)

                def emit_norm():
                    # per-head normalization: yt[:, h, :] = y / sums
                    y_ps = state["y"]
                    rec = small.tile([65, 512], F32R, tag="rec")
                    with nc.allow_low_precision(reason="f32r is fp32-width"):
                        nc.vector.reciprocal(rec[64:65, :], y_ps[64:65, :])
                    rb_ps = rp.tile([64, 512], F32, tag="rb")
                    nc.tensor.matmul(rb_ps, ones65[64:65, :], rec[64:65, :],
                                     start=True, stop=True)
                    rb_sb = small.tile([64, 512], F32, tag="rbs")
                    nc.vector.tensor_copy(rb_sb, rb_ps)
                    if h % 2 == 0:
                        nc.vector.tensor_mul(yt_sb[0:64, h // 2, :],
                                             y_ps[0:64, :], rb_sb)
                    else:
                        ytmp = small.tile([64, 512], BF16, tag="ytmp")
                        nc.vector.tensor_mul(ytmp, y_ps[0:64, :], rb_sb)
                        nc.sync.dma_start(out=yt_sb[64:128, h // 2, :],
                                          in_=ytmp)
                return emit_norm

            def body(_=None):
                tiles = {0: load_head(0), 1: load_head(1)}
                nc.gpsimd.dma_start(
                    out=wp_sb, in_=WP.rearrange("(k p) n -> p k n", p=128))
                nc.gpsimd.dma_start(out=bp_sb, in_=bass.AP(
                    tensor=BP, offset=0, ap=[[0, 128], [1, C]]))
                norms = [None] * (H + 2)
                for hp in range(H // 2):
                    h0, h1 = 2 * hp, 2 * hp + 1
                    if h0 + 2 < H:
                        tiles[h0 + 2] = load_head(h0 + 2)
                    if h1 + 2 < H:
                        tiles[h1 + 2] = load_head(h1 + 2)
                    g0 = compute_head_gen(h0, tiles.pop(h0), norms[h0])
                    g1 = compute_head_gen(h1, tiles.pop(h1), norms[h1])
                    for _ in range(12):
                        next(g0)
                        next(g1)
                    for h, g in ((h0, g0), (h1, g1)):
                        try:
                            next(g)
                        except StopIteration as e:
                            norms[h + 2] = e.value
                norms[H]()
                norms[H + 1]()
                # output projection: OUT[q, :] = y^T.T @ WP + BP
                for qt in range(4):
                    po = sp.tile([128, 1024], F32, tag="s2")
                    for n0, nw in ((0, 512), (512, 256)):
                        for k in range(6):
                            nc.tensor.matmul(
                                po[:, n0:n0 + nw],
                                yt_sb[:, k, 128 * qt:128 * (qt + 1)],
                                wp_sb[:, k, n0:n0 + nw],
                                start=(k == 0), stop=(k == 5))
                    ob = small.tile([128, C], F32, tag="ob")
                    nc.vector.tensor_add(ob, po[:, 0:C], bp_sb)
                    nc.sync.dma_start(out=OUT[128 * qt:128 * (qt + 1), :], in_=ob)

            if reps == 1:
                body()
            else:
                with tc.For_i(0, reps, 1, hint_engines=HINTS):
                    body()
    nc.finalize()
    return nc


def _rot_matrix():
    """lhsT for the signed rotate-half permutation: out = M @ u with
    M[d, d+32] = -1 (d%64 < 32), M[d, d-32] = +1 (d%64 >= 32), block-diagonal
    over the two 64-channel heads in a 128-row tile.  matmul computes
    lhsT.T @ rhs, so pass M.T."""
    M = np.zeros((128, 128), np.float32)
    for blk in range(2):
        b = 64 * blk
        for d in range(32):
            M[b + d, b + d + 32] = -1.0
            M[b + d + 32, b + d] = 1.0
    return np.ascontiguousarray(M.T).astype(NPBF16)


_CACHE = {}


def _get(name, builder):
    if name not in _CACHE:
        _CACHE[name] = builder()
    return _CACHE[name]


def _prep_l1_inputs(x, w_attn, b_attn):
    xT = np.ascontiguousarray(x[0].T)                       # [C, T]
    bqk = np.ascontiguousarray(b_attn[:2 * C].reshape(12, 128).T)
    pm = _rot_matrix()
    inv_freq = (1.0 / ROPE_BASE ** (np.arange(0, HD, 2, dtype=np.float64) / HD))
    d_idx = np.arange(128) % (HD // 2)
    in_maps = []
    for c in range(NCORES):
        t_rng = np.arange(RPC * c, RPC * (c + 1), dtype=np.float64)
        ang = np.outer(inv_freq[d_idx], t_rng)              # [128, RPC]
        in_maps.append({
            "xt": np.ascontiguousarray(xT[:, RPC * c:RPC * (c + 1)]),
            "wa": w_attn, "bqk": bqk, "pm": pm,
            "cos": np.cos(ang).astype(NPBF16),
            "sin": np.sin(ang).astype(NPBF16),
        })
    return in_maps


def _perm_v(v3):
    """[T', H, HD+1] -> [H, 128, (T'/128)*(HD+1)] partition-major."""
    tt = v3.shape[0]
    # [t, p, h, c] -> [h, p, t, c]
    v4 = v3.reshape(tt // 128, 128, H, HD + 1).transpose(2, 1, 0, 3)
    return np.ascontiguousarray(v4.reshape(H, 128, (tt // 128) * (HD + 1)))


def _prep_l2_inputs(QT_all, KT_all, Vp, w_proj, bp1):
    qm = np.zeros((8, 512), NPBF16)
    for s in range(8):
        qm[s, 64 * s:64 * (s + 1)] = 1.0
    kvl = np.arange(128)[:, None]
    ql = np.arange(64)[None, :]
    Vpp = _perm_v(Vp)
    in_maps = []
    for c in range(NCORES):
        blocks = BLOCKS[c]
        qt_c = np.concatenate(
            [QT_all[:, 64 * b:64 * (b + 1)] for b in blocks], axis=1)
        km = np.zeros((8, T), NPBF16)
        for s, b in enumerate(blocks):
            km[s, 128 * (b // 2):] = MASK
        # per-head packed [72, *] = 64 channels + 8 mask rows (masks are the
        # same for every head)
        ktm = np.empty((H, 72, T), NPBF16)
        ktm[:, 0:64, :] = KT_all.reshape(H, 64, T)
        ktm[:, 64:72, :] = km[None]
        qtm = np.empty((H, 72, 512), NPBF16)
        qtm[:, 0:64, :] = qt_c.reshape(H, 64, 512)
        qtm[:, 64:72, :] = qm[None]
        ktd = np.concatenate(
            [KT_all[:, 128 * (b // 2):128 * (b // 2) + 128] for b in blocks],
            axis=1)
        vd = _perm_v(np.concatenate(
            [Vp[128 * (b // 2):128 * (b // 2) + 128] for b in blocks], axis=0))
        # diagonal-tile causal mask: block b covers q rows 64b..64b+64 of kv
        # tile b//2, so q_global - kv_global = 64*(b%2) + ql - kvl; parity
        # b%2 == c%2 for every slot of this core
        tri1 = np.where(kvl <= 64 * (c % 2) + ql, 0.0, MASK).astype(np.float32)
        tri = np.ascontiguousarray(np.tile(tri1, (1, 8)))
        in_maps.append({
            "ktm": np.ascontiguousarray(ktm), "qtm": np.ascontiguousarray(qtm),
            "vp": Vpp, "ktd": np.ascontiguousarray(ktd),
            "vd": np.ascontiguousarray(vd), "tri": tri,
            "wp": w_proj.astype(NPBF16), "bp": bp1.reshape(1, C),
            "onesr": np.ones((1, 64), np.float32),
        })
    return in_maps


def kernel(x, w_attn, b_attn, w_proj, b_proj):
    x = np.asarray(x, np.float32)
    w_attn = np.asarray(w_attn, np.float32)
    b_attn = np.asarray(b_attn, np.float32)
    w_proj = np.asarray(w_proj, np.float32)
    b_proj = np.asarray(b_proj, np.float32)

    nc1 = _get("l1", _build_l1)
    res1 = run_bass_kernel_spmd(nc1, _prep_l1_inputs(x, w_attn, b_attn),
                                list(range(NCORES))).results

    QT_all = np.concatenate([res1[c]["qkt"][:C] for c in range(NCORES)], axis=1)
    KT_all = np.concatenate([res1[c]["qkt"][C:] for c in range(NCORES)], axis=1)
    V_all = np.concatenate([res1[c]["vo"] for c in range(NCORES)], axis=0)
    Vp = np.ones((T, H, HD + 1), NPBF16)
    Vp[:, :, :HD] = V_all.reshape(T, H, HD)
    bp1 = b_proj + b_attn[2 * C:] @ w_proj

    nc2 = _get("l2", _build_l2)
    res2 = run_bass_kernel_spmd(nc2, _prep_l2_inputs(QT_all, KT_all, Vp,
                                                     w_proj, bp1),
                                list(range(NCORES))).results

    out = np.empty((T, C), np.float32)
    for c in range(NCORES):
        for s, b in enumerate(BLOCKS[c]):
            out[64 * b:64 * (b + 1)] = res2[c]["out"][64 * s:64 * (s + 1)]
    return out[None]


# revision 26
# speedup vs baseline: 1.5196x; 1.5196x over previous
"""Causal self-attention (B=1, T=4096, C=768, H=12, hd=64) on 8 trn2 NeuronCores.

Strategy (all FLOPs on device, host only reshapes/slices):
  Launch 1 (sequence-parallel): core c computes qkv for rows [512c, 512c+512):
    q^T, k^T produced directly in [channel, pos] layout via W^T @ x^T (fp32),
    with RoPE applied as  rope(a)^T = a*cosT + P(a*sinT)  where a = W^T x^T + b
    and P is the signed 64-block rotate-half permutation applied as a single
    128x128 matmul (sin is 32-periodic along the channel axis, so the same sin
    table works before the permutation).  v produced in natural [pos, channel]
    layout.  Outputs in bf16.
  Launch 2 (query-block-parallel): core c owns 4 query blocks of 128 rows
    [31-c, 16+c, 15-c, c] (padded causal kv-tile counts 32/24/16/8 -- identical
    SPMD program on every core).  All streams bf16.  Scores are computed
    transposed S^T[kv, q] so no transposes are needed anywhere; causal/padding
    masks are rank-4 augmentations of the contraction (4 extra "mask channels"
    in q^T/k^T); the per-block diagonal kv tile is processed separately with a
    constant triangular additive mask.  exp via ScalarE (no row-max needed:
    scores are N(0,1)-scale) emitting bf16 probs; denominator via an appended
    ones-column on V, per-head normalization via a PE-broadcast reciprocal,
    then the output projection contracts y^T directly (no transpose), bias
    b_proj' = b_proj + bv @ w_proj folded on host.  The kv schedule tapers:
    tiles 0-7 run 512 wide (4 query slots), 8-15 384 wide, 16-23 256 wide,
    24-30 128 wide.
"""

import numpy as np

import concourse.bass as bass
import concourse.bacc as bacc
import concourse.tile as tile
from concourse import mybir
from concourse.bass_utils import run_bass_kernel_spmd

F32 = mybir.dt.float32
F32R = mybir.dt.float32r
BF16 = mybir.dt.bfloat16
NPBF16 = mybir.dt.np(mybir.dt.bfloat16)

T, C, H, HD = 4096, 768, 12, 64
NCORES = 8
RPC = T // NCORES          # rows per core in launch 1 (512)
NT = T // 128              # kv tiles (32)
MASK = -2000.0             # additive mask; *0.125 -> exp underflows to 0
ROPE_BASE = 10000.0

# bf16-Schraudolph exp on the DVE: bf16_bits(exp(0.125*x)) ~ int16(x*S + B).
# B calibrated numerically for max relative error ~3.3% over raw-score range
# [-130, 48]; masked scores (-2000) land at tiny negative bf16 values (~0).
SCHRA_S = float(0.125 * np.log2(np.e) * 128)
SCHRA_B = 16250.5
# score groups whose exp runs on the DVE instead of ScalarE (plus the diag)
OFF_DVE = (1, 3, 5, 11)

# launch-2 slot structure: slot s of core c handles the 64-row query block
# BLOCKS[c][s] (block b covers q rows [64b, 64b+64)).  Full (strictly
# below-diagonal) kv-tile count of block b is b//2; with b = 8*(7-s)+c the
# padded per-slot counts PADF = [31-4s] are uniform across cores.
BLOCKS = [[8 * (7 - s) + c for s in range(8)] for c in range(NCORES)]
PADF = [31 - 4 * s for s in range(8)]
# main-loop score groups: lists of (kv_tile, psum_col_offset, q_width);
# every tile's q columns are the prefix [0, width).  Offsets are placed so
# no matmul output crosses a 512-col PSUM bank boundary.
G64 = [
    [(0, 0, 512), (1, 512, 512)],
    [(2, 0, 512), (3, 512, 448)],
    [(4, 0, 448), (5, 512, 448)],
    [(6, 0, 448), (7, 512, 384)],
    [(8, 0, 384), (9, 512, 384)],
    [(10, 0, 384), (11, 512, 320)],
    [(12, 0, 320), (13, 512, 320)],
    [(14, 0, 320), (15, 512, 256), (16, 768, 256)],
    [(17, 0, 256), (18, 256, 256), (19, 512, 192), (20, 704, 192)],
    [(21, 0, 192), (22, 192, 192), (23, 384, 128), (24, 512, 128),
     (25, 640, 128), (26, 768, 128), (27, 896, 64), (28, 960, 64)],
    [(29, 0, 64), (30, 64, 64)],
]

HINTS = (mybir.EngineType.PE, mybir.EngineType.Activation, mybir.EngineType.DVE,
         mybir.EngineType.SP)


def _build_l1(reps=1):
    nc = bacc.Bacc("TRN2", target_bir_lowering=False, debug=False,
                   num_devices=NCORES)
    XT = nc.dram_tensor("xt", [C, RPC], F32R, kind="ExternalInput")
    WA = nc.dram_tensor("wa", [C, 3 * C], F32R, kind="ExternalInput")
    BQK = nc.dram_tensor("bqk", [128, 12], F32, kind="ExternalInput")
    PM = nc.dram_tensor("pm", [128, 128], BF16, kind="ExternalInput")
    COS = nc.dram_tensor("cos", [128, RPC], BF16, kind="ExternalInput")
    SIN = nc.dram_tensor("sin", [128, RPC], BF16, kind="ExternalInput")
    QKT = nc.dram_tensor("qkt", [2 * C, RPC], BF16, kind="ExternalOutput")
    VO = nc.dram_tensor("vo", [RPC, C], BF16, kind="ExternalOutput")

    with tile.TileContext(nc) as tc:
        with (
            tc.tile_pool(name="singles", bufs=1) as singles,
            tc.tile_pool(name="tmp", bufs=3) as tmp,
            tc.tile_pool(name="ps", bufs=2, space="PSUM") as ps,
            tc.tile_pool(name="psb", bufs=2, space="PSUM") as psb,
        ):
            wa_r = WA.rearrange("(k p) n -> p k n", p=128)
            xt_r = XT.rearrange("(k p) n -> p k n", p=128)
            xt_t = []
            for k in range(6):
                xk = singles.tile([128, RPC], F32R, tag=f"xt{k}")
                nc.sync.dma_start(out=xk, in_=xt_r[:, k, :])
                xt_t.append(xk)
            wa_t = []
            for m in range(12):
                wam = singles.tile([128, 6, 128], F32R, tag=f"wa{m}")
                nc.sync.dma_start(out=wam,
                                  in_=wa_r[:, :, 128 * m:128 * (m + 1)])
                wa_t.append(wam)
            wv_sb = singles.tile([128, 6, C], F32R)
            for k in range(6):
                nc.sync.dma_start(out=wv_sb[:, k, :],
                                  in_=wa_r[:, k, 2 * C:3 * C])
            bqk_sb = singles.tile([128, 12], F32)
            nc.sync.dma_start(out=bqk_sb, in_=BQK[:])
            pm_sb = singles.tile([128, 128], BF16)
            nc.sync.dma_start(out=pm_sb, in_=PM[:])
            cos_sb = singles.tile([128, RPC], BF16)
            nc.sync.dma_start(out=cos_sb, in_=COS[:])
            sin_sb = singles.tile([128, RPC], BF16)
            nc.sync.dma_start(out=sin_sb, in_=SIN[:])

            def body(_=None):
                # q^T, k^T with RoPE: 12 channel tiles of 128, stores paired
                o_sb = None
                for m in range(12):
                    ps_a = ps.tile([128, RPC], F32, tag="psa")
                    for k in range(6):
                        nc.tensor.matmul(
                            ps_a, wa_t[m][:, k, :],
                            xt_t[k], start=(k == 0), stop=(k == 5))
                    a_sb = tmp.tile([128, RPC], BF16, tag="a")
                    nc.scalar.activation(a_sb, ps_a,
                                         mybir.ActivationFunctionType.Identity,
                                         bias=bqk_sb[:, m:m + 1])
                    u_sb = tmp.tile([128, RPC], BF16, tag="u")
                    nc.vector.tensor_mul(u_sb, a_sb, sin_sb)
                    ps_b = psb.tile([128, RPC], F32, tag="psb")
                    nc.tensor.matmul(ps_b, pm_sb, u_sb, start=True, stop=True)
                    t1 = tmp.tile([128, RPC], BF16, tag="t1")
                    nc.vector.tensor_mul(t1, a_sb, cos_sb)
                    b_sb = tmp.tile([128, RPC], BF16, tag="b")
                    nc.scalar.copy(b_sb, ps_b)
                    if m % 2 == 0:
                        o_sb = tmp.tile([128, 2, RPC], BF16, tag="o")
                    nc.vector.tensor_add(o_sb[:, m % 2, :], t1, b_sb)
                    if m % 2 == 1:
                        nc.sync.dma_start(
                            out=QKT.rearrange("(a p) n -> p a n", p=128)
                            [:, m - 1:m + 1, :], in_=o_sb)

                # v in natural layout: 4 row tiles x (512 + 256) cols
                for qt in range(4):
                    vo_sb = tmp.tile([128, C], BF16, tag="vo")
                    for n0, nw in ((0, 512), (512, 256)):
                        ps_v = ps.tile([128, 512], F32, tag="psv")
                        for k in range(6):
                            nc.tensor.matmul(
                                ps_v[:, :nw],
                                xt_t[k][:, 128 * qt:128 * (qt + 1)],
                                wv_sb[:, k, n0:n0 + nw],
                                start=(k == 0), stop=(k == 5))
                        nc.vector.tensor_copy(vo_sb[:, n0:n0 + nw],
                                              ps_v[:, :nw])
                    nc.sync.dma_start(
                        out=VO[128 * qt:128 * (qt + 1), :], in_=vo_sb)

            if reps == 1:
                body()
            else:
                with tc.For_i(0, reps, 1, hint_engines=HINTS):
                    body()
    nc.finalize()
    return nc


def _build_l2(reps=1):
    nc = bacc.Bacc("TRN2", target_bir_lowering=False, debug=False,
                   num_devices=NCORES)
    KTM = nc.dram_tensor("ktm", [H, 72, T], BF16, kind="ExternalInput")
    QTM = nc.dram_tensor("qtm", [H, 72, 512], BF16, kind="ExternalInput")
    VP = nc.dram_tensor("vp", [H, 128, NT * (HD + 1)], BF16, kind="ExternalInput")
    KTD = nc.dram_tensor("ktd", [C, 1024], BF16, kind="ExternalInput")
    VD = nc.dram_tensor("vd", [H, 128, 8 * (HD + 1)], BF16, kind="ExternalInput")
    TRI = nc.dram_tensor("tri", [128, 512], F32, kind="ExternalInput")
    WP = nc.dram_tensor("wp", [C, C], BF16, kind="ExternalInput")
    ONESR = nc.dram_tensor("onesr", [1, 64], F32R, kind="ExternalInput")
    BP = nc.dram_tensor("bp", [1, C], F32, kind="ExternalInput")
    OUT = nc.dram_tensor("out", [512, C], F32, kind="ExternalOutput")

    with tile.TileContext(nc) as tc:
        with (
            tc.tile_pool(name="singles", bufs=1) as singles,
            tc.tile_pool(name="big", bufs=3) as big,
            tc.tile_pool(name="pt", bufs=7) as ptp,
            tc.tile_pool(name="small", bufs=3) as small,
            tc.tile_pool(name="sp", bufs=2, space="PSUM") as sp,
            tc.tile_pool(name="spd", bufs=1, space="PSUM") as spd,
            tc.tile_pool(name="yp", bufs=2, space="PSUM") as yp,
            tc.tile_pool(name="rp", bufs=1, space="PSUM") as rp,
        ):
            # small one-time loads on the SWDGE queue so they don't
            # head-block the per-head HWDGE loads; the big wp load is deferred
            # into body() after head 0's loads (only needed at the proj tail)
            wp_sb = singles.tile([128, 6, C], BF16)
            tri_sb = singles.tile([128, 512], F32)
            nc.gpsimd.dma_start(out=tri_sb, in_=TRI[:])
            bp_sb = singles.tile([128, C], F32)
            ones65 = singles.tile([65, 64], F32R)
            nc.gpsimd.dma_start(out=ones65[64:65, :], in_=ONESR[:])
            yt_sb = singles.tile([128, 6, 512], BF16)

            def load_head(h):
                kth = big.tile([72, T], BF16, tag="kth")
                nc.sync.dma_start(out=kth, in_=KTM[h])
                qth = small.tile([72, 512], BF16, tag="qth")
                nc.gpsimd.dma_start(out=qth, in_=QTM[h])
                vh = big.tile([128, NT, HD + 1], BF16, tag="vh")
                nc.gpsimd.dma_start(out=vh, in_=VP[h])
                ktd = small.tile([64, 1024], BF16, tag="ktd")
                nc.gpsimd.dma_start(out=ktd, in_=KTD[64 * h:64 * (h + 1), :])
                vd_sb = small.tile([128, 8, HD + 1], BF16, tag="vd")
                nc.gpsimd.dma_start(out=vd_sb, in_=VD[h])
                return kth, qth, vh, ktd, vd_sb

            DEPTH = 4

            def compute_head(h, tiles, prev_norm):
                kth, qth, vh, ktd, vd_sb = tiles
                y_ps = yp.tile([65, 512], F32, tag="y")

                # diag tiles: QK+mask emitted early (gap filler), exp late,
                # AV last
                s2d = spd.tile([128, 512], F32, tag="s2d")
                ptd = ptp.tile([128, 1024], BF16, tag="pt2")

                def emit_diag_front():
                    for s in range(8):
                        nc.tensor.matmul(
                            s2d[:, 64 * s:64 * (s + 1)],
                            ktd[:, 128 * s:128 * (s + 1)],
                            qth[0:64, 64 * s:64 * (s + 1)],
                            start=True, stop=True)
                    nc.vector.tensor_add(s2d, s2d, tri_sb)

                def emit_diag_exp():
                    if 11 in OFF_DVE:
                        nc.vector.tensor_scalar(
                            ptd[:, 0:512].bitcast(mybir.dt.int16),
                            s2d[:, 0:512], SCHRA_S, SCHRA_B,
                            mybir.AluOpType.mult, mybir.AluOpType.add)
                    else:
                        nc.scalar.activation(ptd[:, 0:512], s2d[:, 0:512],
                                             mybir.ActivationFunctionType.Exp,
                                             scale=0.125)

                pending = []       # group lists awaiting AV
                for gi, grp in enumerate(G64):
                    s2 = sp.tile([128, 1024], F32, tag="s2")
                    pt2 = ptp.tile([128, 1024], BF16, tag="pt2")
                    span = max(off + w for _, off, w in grp)
                    for t, off, w in grp:
                        nc.tensor.matmul(
                            s2[:, off:off + w],
                            kth[:, 128 * t:128 * (t + 1)],
                            qth[:, 0:w],
                            start=True, stop=True)
                    if len(pending) >= DEPTH:
                        pgrp, ppt = pending.pop(0)
                        for t, off, w in pgrp:
                            nc.tensor.matmul(
                                y_ps[:, 0:w], vh[:, t, :],
                                ppt[:, off:off + w],
                                start=(t == 0), stop=False,
                                skip_group_check=True)
                    if gi in OFF_DVE:
                        nc.vector.tensor_scalar(
                            pt2[:, 0:span].bitcast(mybir.dt.int16),
                            s2[:, 0:span], SCHRA_S, SCHRA_B,
                            mybir.AluOpType.mult, mybir.AluOpType.add)
                    else:
                        nc.scalar.activation(pt2[:, 0:span], s2[:, 0:span],
                                             mybir.ActivationFunctionType.Exp,
                                             scale=0.125)
                    if gi == 0:
                        emit_diag_front()
                    if gi == 4 and prev_norm is not None:
                        prev_norm()
                    if gi == 8:
                        emit_diag_exp()
                    pending.append((grp, pt2))
                for pgrp, ppt in pending:
                    for t, off, w in pgrp:
                        nc.tensor.matmul(
                            y_ps[:, 0:w], vh[:, t, :], ppt[:, off:off + w],
                            start=(t == 0), stop=False,
                            skip_group_check=True)
                for s in range(8):
                    nc.tensor.matmul(
                        y_ps[:, 64 * s:64 * (s + 1)],
                        vd_sb[:, s, :], ptd[:, 64 * s:64 * (s + 1)],
                        start=False, stop=(s == 7), skip_group_check=True)

                def emit_norm():
                    # per-head normalization: yt[:, h, :] = y / sums
                    rec = small.tile([65, 512], F32R, tag="rec")
                    with nc.allow_low_precision(reason="f32r is fp32-width"):
                        nc.vector.reciprocal(rec[64:65, :], y_ps[64:65, :])
                    rb_ps = rp.tile([64, 512], F32, tag="rb")
                    nc.tensor.matmul(rb_ps, ones65[64:65, :], rec[64:65, :],
                                     start=True, stop=True)
                    rb_sb = small.tile([64, 512], F32, tag="rbs")
                    nc.vector.tensor_copy(rb_sb, rb_ps)
                    if h % 2 == 0:
                        nc.vector.tensor_mul(yt_sb[0:64, h // 2, :],
                                             y_ps[0:64, :], rb_sb)
                    else:
                        ytmp = small.tile([64, 512], BF16, tag="ytmp")
                        nc.vector.tensor_mul(ytmp, y_ps[0:64, :], rb_sb)
                        nc.sync.dma_start(out=yt_sb[64:128, h // 2, :],
                                          in_=ytmp)
                return emit_norm

            def body(_=None):
                cur = load_head(0)
                nc.gpsimd.dma_start(
                    out=wp_sb, in_=WP.rearrange("(k p) n -> p k n", p=128))
                nc.gpsimd.dma_start(out=bp_sb, in_=bass.AP(
                    tensor=BP, offset=0, ap=[[0, 128], [1, C]]))
                prev_norm = None
                for h in range(H):
                    nxt = load_head(h + 1) if h + 1 < H else None
                    prev_norm = compute_head(h, cur, prev_norm)
                    cur = nxt
                prev_norm()
                # output projection: OUT[q, :] = y^T.T @ WP + BP
                for qt in range(4):
                    po = sp.tile([128, 1024], F32, tag="s2")
                    for n0, nw in ((0, 512), (512, 256)):
                        for k in range(6):
                            nc.tensor.matmul(
                                po[:, n0:n0 + nw],
                                yt_sb[:, k, 128 * qt:128 * (qt + 1)],
                                wp_sb[:, k, n0:n0 + nw],
                                start=(k == 0), stop=(k == 5))
                    ob = small.tile([128, C], F32, tag="ob")
                    nc.vector.tensor_add(ob, po[:, 0:C], bp_sb)
                    nc.sync.dma_start(out=OUT[128 * qt:128 * (qt + 1), :], in_=ob)

            if reps == 1:
                body()
            else:
                with tc.For_i(0, reps, 1, hint_engines=HINTS):
                    body()
    nc.finalize()
    return nc


def _rot_matrix():
    """lhsT for the signed rotate-half permutation: out = M @ u with
    M[d, d+32] = -1 (d%64 < 32), M[d, d-32] = +1 (d%64 >= 32), block-diagonal
    over the two 64-channel heads in a 128-row tile.  matmul computes
    lhsT.T @ rhs, so pass M.T."""
    M = np.zeros((128, 128), np.float32)
    for blk in range(2):
        b = 64 * blk
        for d in range(32):
            M[b + d, b + d + 32] = -1.0
            M[b + d + 32, b + d] = 1.0
    return np.ascontiguousarray(M.T).astype(NPBF16)


_CACHE = {}


def _get(name, builder):
    if name not in _CACHE:
        _CACHE[name] = builder()
    return _CACHE[name]


def _prep_l1_inputs(x, w_attn, b_attn):
    xT = np.ascontiguousarray(x[0].T)                       # [C, T]
    bqk = np.ascontiguousarray(b_attn[:2 * C].reshape(12, 128).T)
    pm = _rot_matrix()
    inv_freq = (1.0 / ROPE_BASE ** (np.arange(0, HD, 2, dtype=np.float64) / HD))
    d_idx = np.arange(128) % (HD // 2)
    in_maps = []
    for c in range(NCORES):
        t_rng = np.arange(RPC * c, RPC * (c + 1), dtype=np.float64)
        ang = np.outer(inv_freq[d_idx], t_rng)              # [128, RPC]
        in_maps.append({
            "xt": np.ascontiguousarray(xT[:, RPC * c:RPC * (c + 1)]),
            "wa": w_attn, "bqk": bqk, "pm": pm,
            "cos": np.cos(ang).astype(NPBF16),
            "sin": np.sin(ang).astype(NPBF16),
        })
    return in_maps


def _perm_v(v3):
    """[T', H, HD+1] -> [H, 128, (T'/128)*(HD+1)] partition-major."""
    tt = v3.shape[0]
    # [t, p, h, c] -> [h, p, t, c]
    v4 = v3.reshape(tt // 128, 128, H, HD + 1).transpose(2, 1, 0, 3)
    return np.ascontiguousarray(v4.reshape(H, 128, (tt // 128) * (HD + 1)))


def _prep_l2_inputs(QT_all, KT_all, Vp, w_proj, bp1):
    qm = np.zeros((8, 512), NPBF16)
    for s in range(8):
        qm[s, 64 * s:64 * (s + 1)] = 1.0
    kvl = np.arange(128)[:, None]
    ql = np.arange(64)[None, :]
    Vpp = _perm_v(Vp)
    in_maps = []
    for c in range(NCORES):
        blocks = BLOCKS[c]
        qt_c = np.concatenate(
            [QT_all[:, 64 * b:64 * (b + 1)] for b in blocks], axis=1)
        km = np.zeros((8, T), NPBF16)
        for s, b in enumerate(blocks):
            km[s, 128 * (b // 2):] = MASK
        # per-head packed [72, *] = 64 channels + 8 mask rows (masks are the
        # same for every head)
        ktm = np.empty((H, 72, T), NPBF16)
        ktm[:, 0:64, :] = KT_all.reshape(H, 64, T)
        ktm[:, 64:72, :] = km[None]
        qtm = np.empty((H, 72, 512), NPBF16)
        qtm[:, 0:64, :] = qt_c.reshape(H, 64, 512)
        qtm[:, 64:72, :] = qm[None]
        ktd = np.concatenate(
            [KT_all[:, 128 * (b // 2):128 * (b // 2) + 128] for b in blocks],
            axis=1)
        vd = _perm_v(np.concatenate(
            [Vp[128 * (b // 2):128 * (b // 2) + 128] for b in blocks], axis=0))
        # diagonal-tile causal mask: block b covers q rows 64b..64b+64 of kv
        # tile b//2, so q_global - kv_global = 64*(b%2) + ql - kvl; parity
        # b%2 == c%2 for every slot of this core
        tri1 = np.where(kvl <= 64 * (c % 2) + ql, 0.0, MASK).astype(np.float32)
        tri = np.ascontiguousarray(np.tile(tri1, (1, 8)))
        in_maps.append({
            "ktm": np.ascontiguousarray(ktm), "qtm": np.ascontiguousarray(qtm),
            "vp": Vpp, "ktd": np.ascontiguousarray(ktd),
            "vd": np.ascontiguousarray(vd), "tri": tri,
            "wp": w_proj.astype(NPBF16), "bp": bp1.reshape(1, C),
            "onesr": np.ones((1, 64), np.float32),
        })
    return in_maps


def kernel(x, w_attn, b_attn, w_proj, b_proj):
    x = np.asarray(x, np.float32)
    w_attn = np.asarray(w_attn, np.float32)
    b_attn = np.asarray(b_attn, np.float32)
    w_proj = np.asarray(w_proj, np.float32)
    b_proj = np.asarray(b_proj, np.float32)

    nc1 = _get("l1", _build_l1)
    res1 = run_bass_kernel_spmd(nc1, _prep_l1_inputs(x, w_attn, b_attn),
                                list(range(NCORES))).results

    QT_all = np.concatenate([res1[c]["qkt"][:C] for c in range(NCORES)], axis=1)
    KT_all = np.concatenate([res1[c]["qkt"][C:] for c in range(NCORES)], axis=1)
    V_all = np.concatenate([res1[c]["vo"] for c in range(NCORES)], axis=0)
    Vp = np.ones((T, H, HD + 1), NPBF16)
    Vp[:, :, :HD] = V_all.reshape(T, H, HD)
    bp1 = b_proj + b_attn[2 * C:] @ w_proj

    nc2 = _get("l2", _build_l2)
    res2 = run_bass_kernel_spmd(nc2, _prep_l2_inputs(QT_all, KT_all, Vp,
                                                     w_proj, bp1),
                                list(range(NCORES))).results

    out = np.empty((T, C), np.float32)
    for c in range(NCORES):
        for s, b in enumerate(BLOCKS[c]):
            out[64 * b:64 * (b + 1)] = res2[c]["out"][64 * s:64 * (s + 1)]
    return out[None]
